# revision 1
# baseline (speedup 1.0000x reference)
"""Distributed Trainium2 Bass kernel for nn_AttentionCircuit (moe_routing).

8 NeuronCores, SPMD (cost-model sim 470 us; clean-window silicon
measurements track sim within ~5%; rel err 5.1e-3 on two input seeds):
  Phase 1 (token-sharded, T=512 tokens/core), all matmuls bf16:
    A^T[n,t] = emb @ x^T        dense on TensorE, per pool (qk, v)
    G^T      = M^T * A^T        M^T = gates pre-scattered by indices
                                (host-side index/gate layout prep);
                                G_V/G_K built in place over A, G_Q separate
    V[t,d]   = G_V^T.T @ w_v    then AllToAll (hides under Q-emit)
    Q^T/K^T[d,t] = w_qk^T @ G   each followed by its own fp8 AllToAll
                                (Q/K ship as e4m3 x16 - scores are tiny,
                                exp scale absorbs /256)
    w/M tables streamed as 1 MB host-swizzled DMAs spread over the
    SP/ACT HWDGE rings + gpsimd SWDGE; embT/w loads quarter-split so
    Tile's subtile dependency tracking unblocks the first consumer
    matmuls ~4x sooner (streams never gate the PE);
    PSUM->SBUF evacuations split between DVE and ScalarE so the V/Q/K
    casts never queue behind gating muls (collective issue latency).
  Phase 2: head-sharded causal attention (2 heads/core, all B):
    S^T = (K^T slice)^T @ Q^T   (transposed scores, K=64, causal block
                                skip; full s-row per t-chunk)
    exp on ScalarE (fused scale); triangular bf16 mask on diag blocks;
    PV with a ones-column in V_hat so the softmax denominator falls out
    of the same matmul; 1/sum column-broadcast via gpsimd
    partition_broadcast.
    Per-head-half AllToAll #2 so the first hides under the second
    half's attention and the second under W_O accumulation.
  Phase 3: token-sharded W_O projection (16 K=64 accum chunks) -> out.

PSUM accumulation fp32 throughout; rel-err gate is 2e-2, bf16 + fp8-QK
lands at ~5.1e-3.
"""

import sys

sys.path.insert(0, "/opt/trn_rl_repo")

import numpy as np
import ml_dtypes

import concourse.bass as bass
import concourse.mybir as mybir
import concourse.tile as tile
from concourse import bacc
from concourse.bass_utils import run_bass_kernel_spmd

BF16 = mybir.dt.bfloat16
FP8 = mybir.dt.float8e4
F32 = mybir.dt.float32
NP_BF16 = ml_dtypes.bfloat16
AF = mybir.ActivationFunctionType
ALU = mybir.AluOpType

B, S, D = 4, 1024, 1024
N, K = 4096, 16
H = 16
DH = D // H            # 64
NCORES = 8
BT = B * S             # 4096 tokens
T = BT // NCORES       # 512 tokens per core
P = 128
NT = N // P            # 32 n-chunks
DT_ = D // P           # 8 d-chunks
TT = T // P            # 4 token tiles per core
SCALE = float(1.0 / np.sqrt(np.float32(DH)))


def build_nc(reps=1):
    nc = bacc.Bacc(None, target_bir_lowering=False)

    xt = nc.declare_dram_parameter("xt", [D, T], BF16, isOutput=False)
    embt_qk = nc.declare_dram_parameter("embt_qk", [D, N], BF16, isOutput=False)
    embt_v = nc.declare_dram_parameter("embt_v", [D, N], BF16, isOutput=False)
    w_qk = nc.declare_dram_parameter("w_qk", [N // 4, 4 * D], BF16, isOutput=False)
    w_v = nc.declare_dram_parameter("w_v", [N // 4, 4 * D], BF16, isOutput=False)
    mt_q = nc.declare_dram_parameter("mt_q", [N // 4, 4 * T], BF16, isOutput=False)
    mt_k = nc.declare_dram_parameter("mt_k", [N // 4, 4 * T], BF16, isOutput=False)
    mt_v = nc.declare_dram_parameter("mt_v", [N // 4, 4 * T], BF16, isOutput=False)
    wo = nc.declare_dram_parameter("wo", [D, D], BF16, isOutput=False)
    tri = nc.declare_dram_parameter("tri", [P, P], BF16, isOutput=False)
    out_ext = nc.declare_dram_parameter("out", [T, D], F32, isOutput=True)

    rg = [list(range(NCORES))]

    with tile.TileContext(nc) as tc:
      for _rep in range(reps):
          with (
              tc.tile_pool(name="persist", bufs=1) as persist,
              tc.tile_pool(name="dram", bufs=1, space="DRAM") as dpool,
              tc.tile_pool(name="stream", bufs=4) as stream,
          ):
              # ---------- resident loads ----------
              xt_t = [persist.tile([P, T], BF16, tag=f"xt{i}", name=f"xt{i}") for i in range(DT_)]
              for i in range(DT_):
                  nc.gpsimd.dma_start(out=xt_t[i][:], in_=xt[i * P:(i + 1) * P, :])
              tri_t = persist.tile([P, P], BF16, tag="tri", name="tri")
              nc.gpsimd.dma_start(out=tri_t[:], in_=tri[:, :])

              # A^T tiles, packed 4 n-chunks per [128, 2048] tile
              A = {
                  pool: [persist.tile([P, 4 * T], BF16, tag=f"A_{pool}_{j}", name=f"A_{pool}_{j}")
                         for j in range(NT // 4)]
                  for pool in ("qk", "v")
              }

              def a_sl(pool, nci, lo=0, width=T):
                  return A[pool][nci // 4][:, (nci % 4) * T + lo:(nci % 4) * T + lo + width]

              # ---------- Phase 1a: activations ----------
              psem_cm = tc.tile_pool(name="ps_all", bufs=1, space="PSUM")
              psem = psem_cm.__enter__()
              with (
                  tc.tile_pool(name="embt", bufs=1) as embtp,
              ):
                  for pool, embt_d in (("qk", embt_qk), ("v", embt_v)):
                      ech = [embtp.tile([P, N], BF16, tag=f"embt{d}", name=f"embt{d}") for d in range(DT_)]
                      for d in range(DT_):
                          eng = nc.sync if d % 2 == 0 else nc.scalar
                          for q in range(4):
                              eng.dma_start(
                                  out=ech[d][:, q * (N // 4):(q + 1) * (N // 4)],
                                  in_=embt_d[d * P:(d + 1) * P,
                                             q * (N // 4):(q + 1) * (N // 4)])
                      for nci in range(NT):
                          ps = psem.tile([P, T], F32, tag=f"eps{nci % 8}",
                                         name="actps")
                          for d in range(DT_):
                              nc.tensor.matmul(
                                  out=ps[:],
                                  lhsT=ech[d][:, nci * P:(nci + 1) * P],
                                  rhs=xt_t[d][:],
                                  start=(d == 0),
                                  stop=(d == DT_ - 1),
                              )
                          if nci % 2 == 0:
                              nc.vector.tensor_copy(out=a_sl(pool, nci), in_=ps[:])
                          else:
                              nc.scalar.activation(out=a_sl(pool, nci),
                                                   in_=ps[:], func=AF.Copy)

              # ---------- Phase 1b: gating + emit ----------
              qt_t = [persist.tile([P, T], FP8, tag=f"qt{d}", name=f"qt{d}") for d in range(DT_)]
              kt_t = [persist.tile([P, T], FP8, tag=f"kt{d}", name=f"kt{d}") for d in range(DT_)]
              vbig = persist.tile([P, TT * D], BF16, tag="vbig", name="vbig")
              v_t = [vbig[:, t * D:(t + 1) * D] for t in range(TT)]

              with (
                  tc.tile_pool(name="emit", bufs=1) as emitp,
                  tc.tile_pool(name="wstream", bufs=5) as ws,
              ):
                  # G_Q gets its own buffer; G_K overwrites A_qk in place
                  # (last reader), G_V overwrites A_v in place. This removes
                  # the G-buffer WAW chain so all gating DVE muls can run
                  # during/right after the act phase.
                  Gq = [emitp.tile([P, 4 * T], BF16, tag=f"G{j}", name=f"G{j}") for j in range(NT // 4)]

                  def gq_sl(nci, lo=0, width=T):
                      return Gq[nci // 4][:, (nci % 4) * T + lo:(nci % 4) * T + lo + width]

                  def build_g(mt_param, pool, out_sl):
                      for ncg in range(NT // 4):
                          m = stream.tile([P, 4 * T], BF16, tag="mt_stream", name="mt_stream")
                          nc.gpsimd.dma_start(
                              out=m[:], in_=mt_param[ncg * P:(ncg + 1) * P, :])
                          for k in range(4):
                              nci = ncg * 4 + k
                              nc.vector.tensor_tensor(
                                  out=out_sl(nci), in0=a_sl(pool, nci),
                                  in1=m[:, k * T:(k + 1) * T], op=ALU.mult)

                  def gk_sl(nci, lo=0, width=T):
                      return a_sl("qk", nci, lo, width)

                  def gv_sl(nci, lo=0, width=T):
                      return a_sl("v", nci, lo, width)

                  build_g(mt_v, "v", lambda n: a_sl("v", n))        # G_V in place
                  # V side: out[t, d] accumulated over n, 8 psum banks (4t x 2half)
                  psv = [psem.tile([P, T], F32, tag=f"eps{i}", name=f"eps{i}") for i in range(8)]
                  for ncg in range(NT // 4):
                      wch = ws.tile([P, 4 * D], BF16, tag="w_stream", name="w_stream")
                      for q in range(4):
                          nc.sync.dma_start(
                              out=wch[:, q * D:(q + 1) * D],
                              in_=w_v[ncg * P:(ncg + 1) * P,
                                      q * D:(q + 1) * D])
                      for k in range(4):
                          nci = ncg * 4 + k
                          for tt_ in range(TT):
                              for hf in range(2):
                                  nc.tensor.matmul(
                                      out=psv[tt_ * 2 + hf][:],
                                      lhsT=gv_sl(nci, lo=tt_ * P, width=P),
                                      rhs=wch[:, k * D + hf * T:k * D + (hf + 1) * T],
                                      start=(nci == 0),
                                      stop=(nci == NT - 1),
                                  )
                  for tt_ in range(TT):
                      for hf in range(2):
                          nc.scalar.activation(
                              out=v_t[tt_][:, hf * T:(hf + 1) * T],
                              in_=psv[tt_ * 2 + hf][:], func=AF.Copy)

                  # ---------- A2A #1b (V) — overlaps score matmuls ----------
                  a1vi = dpool.tile([NCORES, P, T], BF16, tag="a1vi", name="a1vi")
                  a1vo = dpool.tile([NCORES, P, T], BF16, tag="a1vo", name="a1vo")
                  vb4 = vbig.rearrange("p (k c) -> p k c", k=TT)
                  for j in range(NCORES):
                      nc.scalar.dma_start(
                          out=a1vi[j, :, :].rearrange("p (k c) -> p k c", k=TT),
                          in_=vb4[:, :, j * P:(j + 1) * P])
                  nc.gpsimd.collective_compute(
                      "AllToAll", ALU.bypass, replica_groups=rg,
                      ins=[a1vi.opt()], outs=[a1vo.opt()])

                  build_g(mt_q, "qk", gq_sl)                        # G_Q -> Gq
                  build_g(mt_k, "qk", lambda n: a_sl("qk", n))      # G_K in place
                  # ---------- A2A #1a/#1a' (Q^T after Q-emit, K^T after
                  # K-emit) — each hides under the next emit pass ----------
                  a1qi = dpool.tile([NCORES, P, T], FP8, tag="a1qi", name="a1qi")
                  a1qo = dpool.tile([NCORES, P, T], FP8, tag="a1qo", name="a1qo")
                  a1ki = dpool.tile([NCORES, P, T], FP8, tag="a1ki", name="a1ki")
                  a1ko = dpool.tile([NCORES, P, T], FP8, tag="a1ko", name="a1ko")
                  # Q then K: out^T[d, t] accumulated over n, 8 psum banks
                  for g_sl_, out_tiles in ((gq_sl, qt_t), (gk_sl, kt_t)):
                      psq = [psem.tile([P, T], F32, tag=f"eps{d}", name=f"eps{d}") for d in range(DT_)]
                      for ncg in range(NT // 4):
                          wch = ws.tile([P, 4 * D], BF16, tag="w_stream", name="w_stream")
                          for q in range(4):
                              nc.sync.dma_start(
                                  out=wch[:, q * D:(q + 1) * D],
                                  in_=w_qk[ncg * P:(ncg + 1) * P,
                                           q * D:(q + 1) * D])
                          for k in range(4):
                              nci = ncg * 4 + k
                              for d in range(DT_):
                                  nc.tensor.matmul(
                                      out=psq[d][:],
                                      lhsT=wch[:, k * D + d * P:k * D + (d + 1) * P],
                                      rhs=g_sl_(nci),
                                      start=(nci == 0),
                                      stop=(nci == NT - 1),
                                  )
                      for d in range(DT_):
                          nc.vector.tensor_scalar_mul(
                              out_tiles[d][:], psq[d][:], 16.0)
                      bi, bo = (a1qi, a1qo) if out_tiles is qt_t else (a1ki, a1ko)
                      for j in range(NCORES):
                          nc.scalar.dma_start(out=bi[j, :, :],
                                              in_=out_tiles[j][:])
                      nc.gpsimd.collective_compute(
                          "AllToAll", ALU.bypass, replica_groups=rg,
                          ins=[bi.opt()], outs=[bo.opt()])


              psem_cm.__exit__(None, None, None)
              qt_full = persist.tile([P, BT], FP8, tag="qt_full", name="qt_full")
              kt_full = persist.tile([P, BT], FP8, tag="kt_full", name="kt_full")
              # v_full[i]: [128t, 4*128d] for src i, col-block k = t-tile k
              v_full = [persist.tile([P, T], BF16, tag=f"vf{i}", name=f"vf{i}") for i in range(NCORES)]
              for i in range(NCORES):
                  nc.scalar.dma_start(out=qt_full[:, i * T:(i + 1) * T], in_=a1qo[i, :, :])
                  nc.scalar.dma_start(out=kt_full[:, i * T:(i + 1) * T], in_=a1ko[i, :, :])
                  nc.scalar.dma_start(out=v_full[i][:], in_=a1vo[i, :, :])

              # ---------- Phase 2: causal attention, 2 heads (h'=0,1) ----------
              # hp-outer so each head-half's att pieces ship in their own
              # AllToAll; A2A#2a hides under hp=1 attention, A2A#2b under
              # the first half of W_O accumulation.
              a2i = [dpool.tile([NCORES, DH, T], BF16, tag=f"a2i{hp}", name=f"a2i{hp}")
                     for hp in range(2)]
              a2o = [dpool.tile([NCORES, DH, T], BF16, tag=f"a2o{hp}", name=f"a2o{hp}")
                     for hp in range(2)]

              with (
                  tc.tile_pool(name="attn", bufs=2) as attnp,
                  tc.tile_pool(name="pt_pool", bufs=2) as ptp,
                  tc.tile_pool(name="ps_s", bufs=3, space="PSUM") as pss,
                  tc.tile_pool(name="ps_att", bufs=2, space="PSUM") as psatt,
              ):
                  ones_t = attnp.tile([1, DH], BF16, tag="ones", name="ones")
                  nc.vector.memset(ones_t[:], 1.0)

                  for hp in range(2):
                      for b in range(B):
                          # V_hat tiles for this (b, h'): 8 t-chunks [128, 65]
                          vhat = []
                          for jj in range(8):
                              i_src = 2 * b + jj // 4
                              k_ = jj % 4
                              vh = attnp.tile([P, DH + 1], BF16, tag=f"vhat{jj}", name=f"vhat{jj}")
                              nc.vector.tensor_copy(
                                  out=vh[:, 0:DH],
                                  in_=v_full[i_src][:, k_ * P + hp * DH:
                                                    k_ * P + hp * DH + DH])
                              nc.vector.memset(vh[:, DH:DH + 1], 1.0)
                              vhat.append(vh)

                          qt_b = qt_full[hp * DH:(hp + 1) * DH,
                                         b * S:(b + 1) * S]   # [64, 1024]
                          kt_b = kt_full[hp * DH:(hp + 1) * DH,
                                         b * S:(b + 1) * S]

                          # scores+exp for all 8 t-chunks, full s-row each
                          pts = []
                          for j in range(8):
                              t0 = j * P
                              s0 = t0            # causal: s >= t
                              ps_s = pss.tile([P, S], F32, tag="s", name="s")
                              lo = s0
                              while lo < S:      # MMs of <=512 free
                                  hi = min(lo + T, (lo // T) * T + T)
                                  nc.tensor.matmul(
                                      out=ps_s[:, lo:hi],
                                      lhsT=kt_b[:, t0:t0 + P],
                                      rhs=qt_b[:, lo:hi],
                                      start=True, stop=True)
                                  lo = hi
                              pt = ptp.tile([P, S], BF16, tag=f"pt{j}", name=f"pt{j}")
                              if s0 % T > 0:
                                  nc.vector.memset(
                                      pt[:, (s0 // T) * T:s0], 0.0)
                              nc.scalar.activation(
                                  out=pt[:, s0:S], in_=ps_s[:, s0:S],
                                  func=AF.Exp, scale=SCALE / 256.0)
                              nc.vector.tensor_tensor(
                                  out=pt[:, s0:s0 + P],
                                  in0=pt[:, s0:s0 + P],
                                  in1=tri_t[:], op=ALU.mult)
                              pts.append(pt)

                          for h2 in range(2):    # s-half PV accumulation
                              ps_a = psatt.tile([DH + 1, T], F32, tag="att", name="att")
                              njc = (h2 + 1) * 4
                              for j in range(njc):
                                  nc.tensor.matmul(
                                      out=ps_a[:],
                                      lhsT=vhat[j][:],
                                      rhs=pts[j][:, h2 * T:(h2 + 1) * T],
                                      start=(j == 0),
                                      stop=(j == njc - 1))
                              # normalize: att[0:64] * (1/sum) broadcast via PE
                              rec = attnp.tile([1, T], BF16, tag="rec", name="rec")
                              with nc.allow_low_precision(
                                      reason="softmax denom recip in bf16"):
                                  nc.vector.reciprocal(
                                      out=rec[:], in_=ps_a[DH:DH + 1, :])
                              bc_sb = attnp.tile([DH, T], BF16, tag="bc_sb", name="bc_sb")
                              nc.gpsimd.partition_broadcast(bc_sb[:], rec[:])
                              att_sb = attnp.tile([DH, T], BF16, tag="att_sb", name="att_sb")
                              nc.vector.tensor_tensor(
                                  out=att_sb[:], in0=ps_a[0:DH, :], in1=bc_sb[:],
                                  op=ALU.mult)
                              nc.scalar.dma_start(
                                  out=a2i[hp][2 * b + h2, :, :],
                                  in_=att_sb[:])
                      # ship this head-half's pieces
                      nc.gpsimd.collective_compute(
                          "AllToAll", ALU.bypass, replica_groups=rg,
                          ins=[a2i[hp].opt()], outs=[a2o[hp].opt()])

              # ---------- W_O (accumulate a2o[0] chunks, then a2o[1]) ----------
              with (
                  tc.tile_pool(name="wop", bufs=1) as wop,
                  tc.tile_pool(name="ps_wo", bufs=1, space="PSUM") as pswo,
              ):
                  woin = [[wop.tile([DH, T], BF16, tag=f"woin{hp}_{i}", name=f"woin{hp}_{i}")
                           for i in range(NCORES)] for hp in range(2)]
                  wo_t = [[wop.tile([DH, D], BF16, tag=f"wo{hp}_{i}", name=f"wo{hp}_{i}")
                           for i in range(NCORES)] for hp in range(2)]
                  for i in range(NCORES):
                      nc.scalar.dma_start(out=woin[0][i][:], in_=a2o[0][i, :, :])
                      for hp in range(2):
                          d0 = i * P + hp * DH
                          nc.sync.dma_start(out=wo_t[hp][i][:],
                                            in_=wo[d0:d0 + DH, :])
                  for i in range(NCORES):
                      nc.scalar.dma_start(out=woin[1][i][:], in_=a2o[1][i, :, :])

                  pso = [pswo.tile([P, T], F32, tag=f"wops{i}", name=f"wops{i}")
                         for i in range(8)]   # (t-tile, hf)
                  for i in range(NCORES):
                      for tt_ in range(TT):
                          for hf in range(2):
                              nc.tensor.matmul(
                                  out=pso[tt_ * 2 + hf][:],
                                  lhsT=woin[0][i][:, tt_ * P:(tt_ + 1) * P],
                                  rhs=wo_t[0][i][:, hf * T:(hf + 1) * T],
                                  start=(i == 0),
                                  stop=False,
                              )
                  for tt_ in range(TT):
                      for i in range(NCORES):
                          for hf in range(2):
                              nc.tensor.matmul(
                                  out=pso[tt_ * 2 + hf][:],
                                  lhsT=woin[1][i][:, tt_ * P:(tt_ + 1) * P],
                                  rhs=wo_t[1][i][:, hf * T:(hf + 1) * T],
                                  start=False,
                                  stop=(i == NCORES - 1),
                              )
                      out_sb = wop.tile([P, D], F32, tag="out_sb",
                                        name="out_sb", bufs=2)
                      for hf in range(2):
                          nc.scalar.activation(
                              out=out_sb[:, hf * T:(hf + 1) * T],
                              in_=pso[tt_ * 2 + hf][:], func=AF.Copy)
                      nc.sync.dma_start(
                          out=out_ext[tt_ * P:(tt_ + 1) * P, :], in_=out_sb[:])

    nc.finalize()
    return nc


_NC_CACHE = {}


def _get_nc():
    if "nc" not in _NC_CACHE:
        _NC_CACHE["nc"] = build_nc()
    return _NC_CACHE["nc"]


def _scatter_gates(idx, gate):
    """[N, BT] matrix M^T with M^T[n, t] = sum_k gate[t,k]*(idx[t,k]==n)."""
    mt = np.zeros((N, BT), np.float32)
    t_idx = np.repeat(np.arange(BT, dtype=np.int64), K)
    np.add.at(mt, (idx.reshape(-1).astype(np.int64), t_idx), gate.reshape(-1))
    return mt


def prepare_in_maps(inputs):
    x = np.asarray(inputs["x"], np.float32).reshape(BT, D)
    xt_full = np.ascontiguousarray(x.T).astype(NP_BF16)           # [D, BT]
    embt_qk = np.ascontiguousarray(
        np.asarray(inputs["qk_emb"], np.float32).T).astype(NP_BF16)
    embt_v = np.ascontiguousarray(
        np.asarray(inputs["v_emb"], np.float32).T).astype(NP_BF16)
    def _swz(w, cols):
        return np.ascontiguousarray(
            w.reshape(N // 512, 4, P, cols).transpose(0, 2, 1, 3)
            .reshape(N // 4, 4 * cols))

    w_qk = _swz(np.asarray(inputs["qk_w"], np.float32), D).astype(NP_BF16)
    w_v = _swz(np.asarray(inputs["v_w"], np.float32), D).astype(NP_BF16)
    wo = np.asarray(inputs["W_O"], np.float32).astype(NP_BF16)
    tri = np.triu(np.ones((P, P), np.float32)).astype(NP_BF16)

    mts = {}
    for side, gk, ik in (("q", "tk_g_Q", "tk_i_Q"),
                         ("k", "tk_g_K", "tk_i_K"),
                         ("v", "tk_g_V", "tk_i_V")):
        mts[side] = _scatter_gates(
            np.asarray(inputs[ik]).reshape(BT, K),
            np.asarray(inputs[gk], np.float32).reshape(BT, K)).astype(NP_BF16)

    in_maps = []
    for c in range(NCORES):
        sl = slice(c * T, (c + 1) * T)
        in_maps.append({
            "xt": np.ascontiguousarray(xt_full[:, sl]),
            "embt_qk": embt_qk,
            "embt_v": embt_v,
            "w_qk": w_qk,
            "w_v": w_v,
            "mt_q": _swz(np.ascontiguousarray(mts["q"][:, sl]), T),
            "mt_k": _swz(np.ascontiguousarray(mts["k"][:, sl]), T),
            "mt_v": _swz(np.ascontiguousarray(mts["v"][:, sl]), T),
            "wo": wo,
            "tri": tri,
        })
    return in_maps


def run(inputs, **kw):
    in_maps = prepare_in_maps(inputs)
    nc = _get_nc()
    res = run_bass_kernel_spmd(nc, in_maps, core_ids=list(range(NCORES)), **kw)
    out = np.concatenate(
        [np.asarray(r["out"], np.float32) for r in res.results], axis=0)
    return out.reshape(B, S, D), res


def kernel(**inputs):
    out, _ = run(inputs)
    return out


def time_exec(inputs, iters=8):
    """Steady-state wall-clock per-exec time (ns) with resident device inputs.

    Mirrors bass2jax.run_bass_via_pjrt's multi-core path but keeps the jitted
    callable and device-resident inputs so repeated calls measure execute
    dispatch + HW time only (no H2D re-transfer, no donation)."""
    import time as _time
    import jax
    from jax.sharding import Mesh, PartitionSpec, NamedSharding
    from jax.experimental.shard_map import shard_map
    from concourse import bass2jax, mybir as mb
    from concourse.bass2jax import _bass_exec_p, partition_id_tensor, \
        install_neuronx_cc_hook

    install_neuronx_cc_hook()
    nc = _get_nc()
    in_maps = prepare_in_maps(inputs)
    n_cores = NCORES

    partition_name = nc.partition_id_tensor.name if nc.partition_id_tensor else None
    in_names, out_names, out_avals = [], [], []
    for alloc in nc.m.functions[0].allocations:
        if not isinstance(alloc, mb.MemoryLocationSet):
            continue
        name = alloc.memorylocations[0].name
        if alloc.kind == "ExternalInput":
            if name != partition_name:
                in_names.append(name)
        elif alloc.kind == "ExternalOutput":
            out_names.append(name)
            out_avals.append(jax.core.ShapedArray(
                tuple(alloc.tensor_shape), mb.dt.np(alloc.dtype)))
    n_params = len(in_names)
    all_names = in_names + out_names
    if partition_name is not None:
        all_names = all_names + [partition_name]

    def _body(*args):
        operands = list(args)
        if partition_name is not None:
            operands.append(partition_id_tensor())
        return tuple(_bass_exec_p.bind(
            *operands,
            out_avals=tuple(out_avals),
            in_names=tuple(all_names),
            out_names=tuple(out_names),
            lowering_input_output_aliases=(),
            sim_require_finite=True,
            sim_require_nnan=True,
            nc=nc,
        ))

    devices = jax.devices()[:n_cores]
    mesh = Mesh(np.asarray(devices), ("core",))
    spec = PartitionSpec("core")
    in_specs = (spec,) * (n_params + len(out_names))
    out_specs = (spec,) * len(out_names)
    fn = jax.jit(shard_map(_body, mesh=mesh, in_specs=in_specs,
                           out_specs=out_specs, check_rep=False),
                 keep_unused=True)

    sharding = NamedSharding(mesh, spec)
    dev_in = [
        jax.device_put(
            np.concatenate([np.asarray(in_maps[c][nm]) for c in range(n_cores)], 0),
            sharding)
        for nm in in_names
    ]
    dev_zero = [
        jax.device_put(
            np.zeros((n_cores * av.shape[0], *av.shape[1:]), av.dtype), sharding)
        for av in out_avals
    ]

    # warmup / compile
    outs = fn(*dev_in, *dev_zero)
    jax.block_until_ready(outs)
    # pipelined: queue all execs, block once — amortizes tunnel RTT
    t0 = _time.perf_counter()
    all_outs = [fn(*dev_in, *dev_zero) for _ in range(iters)]
    jax.block_until_ready(all_outs)
    dt_pipe = (_time.perf_counter() - t0) / iters
    # serial for reference
    t0 = _time.perf_counter()
    outs = fn(*dev_in, *dev_zero)
    jax.block_until_ready(outs)
    dt_serial = _time.perf_counter() - t0
    return dt_pipe * 1e9, dt_serial * 1e9



# revision 23
# speedup vs baseline: 1.8265x; 1.8265x over previous
"""Distributed Trainium2 Bass kernel for nn_AttentionCircuit (moe_routing).

8 NeuronCores, SPMD, fully token-sharded (T=512 tokens/core, batch b=c//2):
  Phase 1: dense A^T = emb @ x^T per pool; G^T = M^T * A^T (gates
    pre-scattered host-side); emits V[t,d] / Q^T,K^T[d,t].
    qk-side (A_qk, Q emit, K emit) runs fp8e4 DoubleRow (2 contraction
    chunks per matmul) with a host-side x16 scale on emb/w absorbed by
    the exp(score/256) trick; V side stays bf16 for accuracy.
  Phase 2: causal attention stays token-local: a core's queries only
    need the previous 512 tokens' K/V, which live on the paired core.
    Pair ReduceScatters (groups [[0,1],[2,3],..]) move K^T (fp8) and
    V-with-ones-columns (bf16) from even->odd cores; sends are
    parity-masked (x par), slot0 carries zeros, so even cores receive
    exact zeros: zero K -> exp(0)=1 but zero V AND zero ones-column
    kill both numerator and softmax denominator -- no masking ops.
    Attention runs in two passes per head (own keys first, neighbor
    keys after) so the V ReduceScatter hides under Q emit + pass A.
  Phase 3: token-local W_O projection -> out [T, D] f32.
"""

import sys

sys.path.insert(0, "/opt/trn_rl_repo")

import numpy as np
import ml_dtypes

import concourse.bass as bass
import concourse.mybir as mybir
import concourse.tile as tile
from concourse import bacc
from concourse.bass_utils import run_bass_kernel_spmd

BF16 = mybir.dt.bfloat16
FP8 = mybir.dt.float8e4
F32 = mybir.dt.float32
NP_BF16 = ml_dtypes.bfloat16
NP_FP8 = ml_dtypes.float8_e4m3
AF = mybir.ActivationFunctionType
ALU = mybir.AluOpType
DR = mybir.MatmulPerfMode.DoubleRow

B, S, D = 4, 1024, 1024
N, K = 4096, 16
H = 16
DH = D // H            # 64
NCORES = 8
BT = B * S             # 4096 tokens
T = BT // NCORES       # 512 tokens per core
P = 128
NT = N // P            # 32 n-chunks
DT_ = D // P           # 8 d-chunks
TT = T // P            # 4 token tiles per core
W65 = DH + 1           # per-head V width incl ones column
VW = H * W65           # 1040 columns per v t-chunk
SCALE = float(1.0 / np.sqrt(np.float32(DH)))


def build_nc(reps=1, stop=7):
    nc = bacc.Bacc(None, target_bir_lowering=False)

    xt = nc.declare_dram_parameter("xt", [D, T], BF16, isOutput=False)
    xt8 = nc.declare_dram_parameter("xt8", [4 * P, 2 * T], FP8, isOutput=False)
    eqk8 = nc.declare_dram_parameter("eqk8", [4 * P, 2 * N], FP8, isOutput=False)
    embt_v = nc.declare_dram_parameter("embt_v", [D, N], BF16, isOutput=False)
    w8 = nc.declare_dram_parameter("w8", [N // 2, D * 2], FP8, isOutput=False)
    w_v = nc.declare_dram_parameter("w_v", [N // 4, 4 * D], BF16, isOutput=False)
    mt_q = nc.declare_dram_parameter("mt_q", [N // 4, 4 * T], FP8, isOutput=False)
    mt_k = nc.declare_dram_parameter("mt_k", [N // 4, 4 * T], FP8, isOutput=False)
    mt_v = nc.declare_dram_parameter("mt_v", [N // 4, 4 * T], BF16, isOutput=False)
    wo = nc.declare_dram_parameter("wo", [D, D], BF16, isOutput=False)
    tri = nc.declare_dram_parameter("tri", [P, P], BF16, isOutput=False)
    par = nc.declare_dram_parameter("par", [P, 1], F32, isOutput=False)
    z8 = nc.declare_dram_parameter("z8", [P, DT_ * T], FP8, isOutput=False)
    z16 = nc.declare_dram_parameter("z16", [P, TT * VW], BF16, isOutput=False)
    out_ext = nc.declare_dram_parameter("out", [T, D], F32, isOutput=True)

    rg2 = [[2 * i, 2 * i + 1] for i in range(NCORES // 2)]

    class StopPhase(Exception):
        pass

    with tile.TileContext(nc) as tc:
      for _rep in range(reps):
        try:
          with (
              tc.tile_pool(name="persist", bufs=1) as persist,
              tc.tile_pool(name="dram", bufs=1, space="DRAM") as dpool,
          ):
            # ---------- resident loads (gpsimd SWDGE; xt8 first: A_qk
            # needs it immediately, xt only by A_v) ----------
            tri_t = persist.tile([P, P], BF16, tag="tri", name="tri")
            nc.scalar.dma_start(out=tri_t[:], in_=tri[:, :])
            par_t = persist.tile([P, 1], F32, tag="par", name="par")
            nc.scalar.dma_start(out=par_t[:], in_=par[:, :])

            # collective buffers; slot0 = zeros (DRAM->DRAM from host zeros)
            rsk_in = dpool.tile([2, P, DT_ * T], FP8, tag="rski", name="rski")
            rsk_out = dpool.tile([P, DT_ * T], FP8, tag="rsko", name="rsko")
            rsv_in = dpool.tile([2, P, TT * VW], BF16, tag="rsvi", name="rsvi")
            rsv_out = dpool.tile([P, TT * VW], BF16, tag="rsvo", name="rsvo")
            xt_t = [persist.tile([P, T], BF16, tag=f"xt{i}", name=f"xt{i}")
                    for i in range(DT_)]
            for i in range(DT_):
                nc.gpsimd.dma_start(out=xt_t[i][:], in_=xt[i * P:(i + 1) * P, :])
            nc.gpsimd.dma_start(out=rsk_in[0, :, :], in_=z8[:, :])
            nc.gpsimd.dma_start(out=rsv_in[0, :, :], in_=z16[:, :])

            # A_qk tiles (fp8: score path tolerates the quantization)
            Abig = [persist.tile([P, 8 * T], FP8, tag=f"A{j}", name=f"A{j}")
                    for j in range(NT // 8)]

            def a_sl(nci, lo=0, width=T):
                return Abig[nci // 8][:, (nci % 8) * T + lo:
                                      (nci % 8) * T + lo + width]

            Gv = [persist.tile([P, 8 * T], BF16, tag=f"Gv{j}", name=f"Gv{j}")
                  for j in range(NT // 8)]

            def gv_sl(nci, lo=0, width=T):
                return Gv[nci // 8][:, (nci % 8) * T + lo:
                                    (nci % 8) * T + lo + width]

            def g_sl(G, nci):
                return G[nci // 8][:, (nci % 8) * T:(nci % 8 + 1) * T]

            def g_pair(G, pg):
                nci = 2 * pg
                v = G[nci // 8][:, (nci % 8) * T:(nci % 8 + 2) * T]
                return v.rearrange("p (two f) -> p two f", two=2)

            kt_big = persist.tile([P, DT_ * T], FP8, tag="kt", name="kt")
            # snd_k doubles as qt_big later (send is consumed long before
            # Q emit writes it)
            qt_big = persist.tile([P, DT_ * T], FP8, tag="qt", name="qt")
            snd_k = qt_big

            psem_cm = tc.tile_pool(name="ps_all", bufs=1, space="PSUM")
            psem = psem_cm.__enter__()

            # ---------- Phase 1a: A_qk (fp8 DoubleRow); eqk on sync ----------
            with tc.tile_pool(name="eqk", bufs=2) as eqkp:
                xt8_t = [eqkp.tile([P, 2 * T], FP8, tag=f"xt8{c}",
                                   name=f"xt8{c}", bufs=1) for c in range(4)]
                for c in range(4):
                    nc.scalar.dma_start(out=xt8_t[c][:],
                                        in_=xt8[c * P:(c + 1) * P, :])
                for q in range(4):
                    ech = [eqkp.tile([P, 2 * P * 8], FP8, tag=f"eq{c}",
                                     name=f"eq{c}") for c in range(4)]
                    for c in range(4):
                        nc.sync.dma_start(
                            out=ech[c][:],
                            in_=eqk8[c * P:(c + 1) * P,
                                     q * 2048:(q + 1) * 2048])
                    for nn in range(8):
                        nci = q * 8 + nn
                        ps = psem.tile([P, T], F32, tag=f"eps{nci % 8}",
                                       name="actps")
                        for c in range(4):
                            nc.tensor.matmul(
                                out=ps[:],
                                lhsT=ech[c][:, nn * 256:(nn + 1) * 256]
                                .rearrange("p (two f) -> p two f", two=2),
                                rhs=xt8_t[c][:]
                                .rearrange("p (two f) -> p two f", two=2),
                                start=(c == 0), stop=(c == 3),
                                perf_mode=DR,
                            )
                        nc.vector.tensor_copy(out=a_sl(nci), in_=ps[:])

            with tc.tile_pool(name="mstream", bufs=2) as ms:
                # w8 loaded once, resident through Q emit (8 x [P, 4096])
                w8r_cm = tc.tile_pool(name="w8r", bufs=1)
                w8r = w8r_cm.__enter__()
                w8t = [w8r.tile([P, 2 * 2 * D], FP8, tag=f"w8{i}",
                                name=f"w8{i}") for i in range(NT // 4)]
                for i in range(NT // 4):
                    for half in range(2):
                        nc.sync.dma_start(
                            out=w8t[i][:, half * 2 * D:(half + 1) * 2 * D],
                            in_=w8[(i * 2 + half) * P:
                                   (i * 2 + half + 1) * P, :])

                def w8_sl(pg, dd):
                    return w8t[pg // 2][:, (pg % 2) * 2 * D + dd * 256:
                                        (pg % 2) * 2 * D + (dd + 1) * 256]

                # ---------- Phase 1c: K emit (DoubleRow) + RS_k ----------
                # G_k pairs built on the fly (DVE idle during emit)
                with tc.tile_pool(name="gkp", bufs=4) as gkp:
                    psq = [psem.tile([P, T], F32, tag=f"eps{d}", name="kps")
                           for d in range(DT_)]
                    mk_ch = None
                    for pg in range(NT // 2):
                        if pg % 2 == 0:
                            mk_ch = ms.tile([P, 4 * T], FP8, tag="mt8",
                                            name="mt8")
                            nc.scalar.dma_start(
                                out=mk_ch[:],
                                in_=mt_k[(pg // 2) * P:(pg // 2 + 1) * P, :])
                        gk = gkp.tile([P, 2 * T], FP8, tag="gk", name="gk")
                        for k in range(2):
                            nci = pg * 2 + k
                            nc.vector.tensor_tensor(
                                out=gk[:, k * T:(k + 1) * T], in0=a_sl(nci),
                                in1=mk_ch[:, (nci % 4) * T:(nci % 4 + 1) * T],
                                op=ALU.mult)
                        for dd in range(DT_):
                            nc.tensor.matmul(
                                out=psq[dd][:],
                                lhsT=w8_sl(pg, dd)
                                .rearrange("p (two f) -> p two f", two=2),
                                rhs=gk[:]
                                .rearrange("p (two f) -> p two f", two=2),
                                start=(pg == 0), stop=(pg == NT // 2 - 1),
                                perf_mode=DR,
                            )
                    # kt evac + masked send on DVE, ahead of G_q in queue
                    for dd in range(DT_):
                        nc.vector.tensor_scalar_mul(
                            kt_big[:, dd * T:(dd + 1) * T], psq[dd][:],
                            1.0 / 16.0)
                    nc.vector.tensor_scalar_mul(snd_k[:], kt_big[:], par_t[:])
                    nc.gpsimd.dma_start(out=rsk_in[1, :, :], in_=snd_k[:])
                    nc.gpsimd.collective_compute(
                        "ReduceScatter", ALU.add, replica_groups=rg2,
                        ins=[rsk_in.opt()], outs=[rsk_out.opt()])

                # embt_v prefetch (quarters 0,1 on scalar) + A_v stream pool
                if stop < 2:
                    raise StopPhase()
                evp_cm = tc.tile_pool(name="ev", bufs=2)
                evp = evp_cm.__enter__()
                ech_v = {}
                for q in range(2):
                    ech_v[q] = [evp.tile([P, 8 * P], BF16, tag=f"ev{d}",
                                         name=f"ev{d}") for d in range(DT_)]
                    for d in range(DT_):
                        nc.scalar.dma_start(
                            out=ech_v[q][d][:],
                            in_=embt_v[d * P:(d + 1) * P,
                                       q * 1024:(q + 1) * 1024])

                # ---------- Phase 1d: A_v (bf16); evac fused with M_v
                # gating: Abig <- A_v (x) m_v = G_v directly ----------
                mv_ch = {}
                for ncg in range(NT // 4):
                    mv_ch[ncg] = ms.tile([P, 4 * T], BF16, tag="mt", name="mt")
                    nc.gpsimd.dma_start(
                        out=mv_ch[ncg][:],
                        in_=mt_v[ncg * P:(ncg + 1) * P, :])
                for q in range(4):
                    if q >= 2:   # quarters 2,3 streamed just in time
                        ech_v[q] = [evp.tile([P, 8 * P], BF16, tag=f"ev{d}",
                                             name=f"ev{d}")
                                    for d in range(DT_)]
                        for d in range(DT_):
                            nc.scalar.dma_start(
                                out=ech_v[q][d][:],
                                in_=embt_v[d * P:(d + 1) * P,
                                           q * 1024:(q + 1) * 1024])
                    for nn in range(8):
                        nci = q * 8 + nn
                        ps = psem.tile([P, T], F32, tag=f"eps{nci % 8}",
                                       name="avps")
                        for d in range(DT_):
                            nc.tensor.matmul(
                                out=ps[:],
                                lhsT=ech_v[q][d][:, nn * P:(nn + 1) * P],
                                rhs=xt_t[d][:],
                                start=(d == 0), stop=(d == DT_ - 1),
                            )
                        nc.vector.tensor_tensor(
                            out=gv_sl(nci), in0=ps[:],
                            in1=mv_ch[nci // 4][:, (nci % 4) * T:
                                                (nci % 4 + 1) * T],
                            op=ALU.mult)
                evp_cm.__exit__(None, None, None)

                # ---------- Phase 1e: V emit + RS_v ----------
                vloc = [persist.tile([P, VW], BF16, tag=f"vl{t_}",
                                     name=f"vl{t_}") for t_ in range(TT)]
                snd_v = persist.tile([P, TT * VW], BF16, tag="sv", name="sv")
                with tc.tile_pool(name="wvs", bufs=2) as wvp:
                    psv = [psem.tile([P, T], F32, tag=f"eps{i}", name="vps")
                           for i in range(8)]
                    for ncg in range(NT // 4):
                        wch = wvp.tile([P, 4 * D], BF16, tag="wv", name="wv")
                        nc.sync.dma_start(
                            out=wch[:], in_=w_v[ncg * P:(ncg + 1) * P, :])
                        for k in range(4):
                            nci = ncg * 4 + k
                            for tt_ in range(TT):
                                for hf in range(2):
                                    nc.tensor.matmul(
                                        out=psv[tt_ * 2 + hf][:],
                                        lhsT=gv_sl(nci, lo=tt_ * P, width=P),
                                        rhs=wch[:, k * D + hf * T:
                                                k * D + (hf + 1) * T],
                                        start=(nci == 0),
                                        stop=(nci == NT - 1),
                                    )
                    for tt_ in range(TT):
                        # ones columns (memset strided view), then data
                        nc.vector.memset(
                            vloc[tt_][:].rearrange(
                                "p (h f) -> p h f", f=W65)[:, :, DH:W65],
                            1.0)
                        for hf in range(2):
                            dst = (vloc[tt_][:, hf * 8 * W65:
                                             (hf + 1) * 8 * W65]
                                   .rearrange("p (h f) -> p h f", f=W65)
                                   [:, :, 0:DH])
                            srcv = (psv[tt_ * 2 + hf][:]
                                    .rearrange("p (h f) -> p h f", f=DH))
                            if (tt_ * 2 + hf) % 2 == 0:
                                nc.scalar.activation(out=dst, in_=srcv,
                                                     func=AF.Copy)
                            else:
                                nc.vector.tensor_copy(out=dst, in_=srcv)
                        nc.vector.tensor_scalar_mul(
                            snd_v[:, tt_ * VW:(tt_ + 1) * VW],
                            vloc[tt_][:], par_t[:])
                    nc.gpsimd.dma_start(out=rsv_in[1, :, :], in_=snd_v[:])
                    nc.gpsimd.collective_compute(
                        "ReduceScatter", ALU.add, replica_groups=rg2,
                        ins=[rsv_in.opt()], outs=[rsv_out.opt()])

                # ---------- Phase 1f: Q emit (DoubleRow, resident w8;
                # G_q pairs built on the fly: DVE is idle here) ----------
                if stop < 3:
                    raise StopPhase()
                with tc.tile_pool(name="gqp", bufs=4) as gqp:
                    psq = [psem.tile([P, T], F32, tag=f"eps{d}", name="qps")
                           for d in range(DT_)]
                    mq_ch = None
                    for pg in range(NT // 2):
                        if pg % 2 == 0:
                            mq_ch = ms.tile([P, 4 * T], FP8, tag="mt8",
                                            name="mt8")
                            nc.scalar.dma_start(
                                out=mq_ch[:],
                                in_=mt_q[(pg // 2) * P:(pg // 2 + 1) * P, :])
                        gq = gqp.tile([P, 2 * T], FP8, tag="gq", name="gq")
                        for k in range(2):
                            nci = pg * 2 + k
                            nc.vector.tensor_tensor(
                                out=gq[:, k * T:(k + 1) * T], in0=a_sl(nci),
                                in1=mq_ch[:, (nci % 4) * T:(nci % 4 + 1) * T],
                                op=ALU.mult)
                        for dd in range(DT_):
                            nc.tensor.matmul(
                                out=psq[dd][:],
                                lhsT=w8_sl(pg, dd)
                                .rearrange("p (two f) -> p two f", two=2),
                                rhs=gq[:]
                                .rearrange("p (two f) -> p two f", two=2),
                                start=(pg == 0), stop=(pg == NT // 2 - 1),
                                perf_mode=DR,
                            )
                    for dd in range(DT_):
                        nc.vector.tensor_scalar_mul(
                            qt_big[:, dd * T:(dd + 1) * T], psq[dd][:],
                            1.0 / 16.0)
                w8r_cm.__exit__(None, None, None)

            # receive DMAs late on sync (after w_v drained; data long ready)
            krecv = persist.tile([P, DT_ * T], FP8, tag="kr", name="kr")
            vrecv = persist.tile([P, TT * VW], BF16, tag="vr", name="vr")
            nc.sync.dma_start(out=krecv[:], in_=rsk_out[:, :])
            nc.sync.dma_start(out=vrecv[:], in_=rsv_out[:, :])

            psem_cm.__exit__(None, None, None)

            # ---------- Phase 2: token-local causal attention ----------
            def kt_sl(src, h, j):
                return src[(h % 2) * DH:(h % 2 + 1) * DH,
                           (h // 2) * T + j * P:(h // 2) * T + (j + 1) * P]

            def qt_sl(h, lo, width):
                return qt_big[(h % 2) * DH:(h % 2 + 1) * DH,
                              (h // 2) * T + lo:(h // 2) * T + lo + width]

            aout = persist.tile([P, DT_ * T], BF16, tag="aout", name="aout")
            pvA = [persist.tile([W65, T], BF16, tag=f"pvA{h}", name=f"pvA{h}")
                   for h in range(H)]

            with (
                tc.tile_pool(name="attn", bufs=2) as attnp,
                tc.tile_pool(name="ptp", bufs=8) as ptp,
                tc.tile_pool(name="wop", bufs=1) as wop,
            ):
                wo_t = [wop.tile([P, D], BF16, tag=f"wo{dd}", name=f"wo{dd}")
                        for dd in range(DT_)]
                for dd in range(DT_):
                    nc.gpsimd.dma_start(out=wo_t[dd][:],
                                        in_=wo[dd * P:(dd + 1) * P, :])

                # pass A: own keys (causal)
                if stop < 4:
                    raise StopPhase()
                psatt_cm = tc.tile_pool(name="psatt", bufs=4, space="PSUM")
                pss = psatt_cm.__enter__()
                pspv_cm = tc.tile_pool(name="pspv", bufs=2, space="PSUM")
                pspv = pspv_cm.__enter__()
                ptsA = {}
                for h in range(H + 1):
                    if h < H:
                        pts = []
                        for jo in range(TT):
                            lo = jo * P
                            ps_s = pss.tile([P, T], F32, tag="s", name="s")
                            nc.tensor.matmul(
                                out=ps_s[:, lo:T],
                                lhsT=kt_sl(kt_big, h, jo),
                                rhs=qt_sl(h, lo, T - lo),
                                start=True, stop=True)
                            pt = ptp.tile([P, T], BF16, tag="pt", name="pt")
                            if lo > 0:
                                nc.vector.memset(pt[:, 0:lo], 0.0)
                            nc.scalar.activation(
                                out=pt[:, lo:T], in_=ps_s[:, lo:T],
                                func=AF.Exp, scale=SCALE / 256.0)
                            nc.vector.tensor_tensor(
                                out=pt[:, lo:lo + P], in0=pt[:, lo:lo + P],
                                in1=tri_t[:], op=ALU.mult)
                            pts.append(pt)
                        ptsA[h] = pts
                    if h >= 1:
                        hp = h - 1
                        pv = pspv.tile([W65, T], F32, tag="pv", name="pv")
                        for jo in range(TT):
                            nc.tensor.matmul(
                                out=pv[:],
                                lhsT=vloc[jo][:, hp * W65:(hp + 1) * W65],
                                rhs=ptsA[hp][jo][:],
                                start=(jo == 0), stop=(jo == TT - 1))
                        nc.scalar.activation(out=pvA[hp][:], in_=pv[:],
                                             func=AF.Copy)
                        del ptsA[hp]

                # pass B: neighbor keys (zeros on even cores) + normalize
                if stop < 5:
                    pspv_cm.__exit__(None, None, None)
                    psatt_cm.__exit__(None, None, None)
                    raise StopPhase()
                ptsB = {}
                for h in range(H + 1):
                    if h < H:
                        pts = []
                        for jn in range(TT):
                            ps_s = pss.tile([P, T], F32, tag="s", name="s")
                            nc.tensor.matmul(
                                out=ps_s[:],
                                lhsT=kt_sl(krecv, h, jn),
                                rhs=qt_sl(h, 0, T),
                                start=True, stop=True)
                            pt = ptp.tile([P, T], BF16, tag="pt", name="pt")
                            nc.scalar.activation(
                                out=pt[:], in_=ps_s[:],
                                func=AF.Exp, scale=SCALE / 256.0)
                            pts.append(pt)
                        ptsB[h] = pts
                    if h < 1:
                        continue
                    h_ = h - 1
                    pv2 = pspv.tile([W65, T], F32, tag="pv2", name="pv2")
                    for jn in range(TT):
                        nc.tensor.matmul(
                            out=pv2[:],
                            lhsT=vrecv[:, jn * VW + h_ * W65:
                                       jn * VW + (h_ + 1) * W65],
                            rhs=ptsB[h_][jn][:],
                            start=(jn == 0), stop=(jn == TT - 1))
                    del ptsB[h_]
                    h = h_
                    pvc = attnp.tile([W65, T], BF16, tag="pvc", name="pvc")
                    if stop < 6:
                        nc.scalar.activation(out=pvc[:], in_=pv2[:],
                                             func=AF.Copy)
                        continue
                    nc.vector.tensor_tensor(out=pvc[:], in0=pv2[:],
                                            in1=pvA[h][:], op=ALU.add)
                    rec = attnp.tile([1, T], BF16, tag="rec", name="rec")
                    with nc.allow_low_precision(
                            reason="softmax denom recip in bf16"):
                        nc.vector.reciprocal(out=rec[:],
                                             in_=pvc[DH:W65, :])
                    bc_sb = attnp.tile([DH, T], BF16, tag="bc", name="bc")
                    nc.gpsimd.partition_broadcast(bc_sb[:], rec[:])
                    att_sb = attnp.tile([DH, T], BF16, tag="att", name="att")
                    nc.vector.tensor_tensor(
                        out=att_sb[:], in0=pvc[0:DH, :], in1=bc_sb[:],
                        op=ALU.mult)
                    nc.gpsimd.dma_start(
                        out=aout[(h % 2) * DH:(h % 2 + 1) * DH,
                                 (h // 2) * T:(h // 2 + 1) * T],
                        in_=att_sb[:])

                pspv_cm.__exit__(None, None, None)
                psatt_cm.__exit__(None, None, None)

                # ---------- Phase 3: W_O ----------
                if stop < 7:
                    raise StopPhase()
                with tc.tile_pool(name="pswo", bufs=1, space="PSUM") as pswo:
                    pso = [pswo.tile([P, T], F32, tag=f"wops{i}",
                                     name=f"wops{i}") for i in range(8)]
                    for dd in range(DT_):
                        for tt_ in range(TT):
                            for hf in range(2):
                                nc.tensor.matmul(
                                    out=pso[tt_ * 2 + hf][:],
                                    lhsT=aout[:, dd * T + tt_ * P:
                                              dd * T + (tt_ + 1) * P],
                                    rhs=wo_t[dd][:, hf * T:(hf + 1) * T],
                                    start=(dd == 0), stop=(dd == DT_ - 1),
                                )
                    for tt_ in range(TT):
                        out_sb = wop.tile([P, D], F32, tag="osb",
                                          name="osb", bufs=2)
                        for hf in range(2):
                            if hf == 0:
                                nc.scalar.activation(
                                    out=out_sb[:, hf * T:(hf + 1) * T],
                                    in_=pso[tt_ * 2 + hf][:], func=AF.Copy)
                            else:
                                nc.vector.tensor_copy(
                                    out=out_sb[:, hf * T:(hf + 1) * T],
                                    in_=pso[tt_ * 2 + hf][:])
                        nc.sync.dma_start(
                            out=out_ext[tt_ * P:(tt_ + 1) * P, :],
                            in_=out_sb[:])

        except StopPhase:
            pass

    nc.finalize()
    return nc


_NC_CACHE = {}


def _get_nc():
    if "nc" not in _NC_CACHE:
        _NC_CACHE["nc"] = build_nc()
    return _NC_CACHE["nc"]


def _scatter_gates(idx, gate):
    """[N, BT] matrix M^T with M^T[n, t] = sum_k gate[t,k]*(idx[t,k]==n)."""
    mt = np.zeros((N, BT), np.float32)
    t_idx = np.repeat(np.arange(BT, dtype=np.int64), K)
    np.add.at(mt, (idx.reshape(-1).astype(np.int64), t_idx),
              gate.reshape(-1))
    return mt


def _swz(w, cols):
    return np.ascontiguousarray(
        w.reshape(N // 512, 4, P, cols).transpose(0, 2, 1, 3)
        .reshape(N // 4, 4 * cols))


def prepare_in_maps(inputs):
    x = np.asarray(inputs["x"], np.float32).reshape(BT, D)
    xt_full = np.ascontiguousarray(x.T)                      # [D, BT] f32
    xt_bf = xt_full.astype(NP_BF16)

    emb16 = np.asarray(inputs["qk_emb"], np.float32).T * 16.0  # [D, N]
    eqk8 = np.ascontiguousarray(
        emb16.reshape(4, 2, P, NT, P).transpose(0, 2, 3, 1, 4)
        .reshape(4 * P, 2 * N)).astype(NP_FP8)
    w16 = np.asarray(inputs["qk_w"], np.float32) * 16.0       # [N, D]
    w8 = np.ascontiguousarray(
        w16.reshape(N // 256, 2, P, DT_, P).transpose(0, 2, 3, 1, 4)
        .reshape(N // 2, 2 * D)).astype(NP_FP8)

    embt_v = np.ascontiguousarray(
        np.asarray(inputs["v_emb"], np.float32).T).astype(NP_BF16)
    w_v = _swz(np.asarray(inputs["v_w"], np.float32), D).astype(NP_BF16)
    wo = np.asarray(inputs["W_O"], np.float32).astype(NP_BF16)
    tri = np.triu(np.ones((P, P), np.float32)).astype(NP_BF16)
    z8 = np.zeros((P, DT_ * T), NP_FP8)
    z16 = np.zeros((P, TT * VW), NP_BF16)

    mts = {}
    for side, gk, ik in (("q", "tk_g_Q", "tk_i_Q"),
                         ("k", "tk_g_K", "tk_i_K"),
                         ("v", "tk_g_V", "tk_i_V")):
        mts[side] = _scatter_gates(
            np.asarray(inputs[ik]).reshape(BT, K),
            np.asarray(inputs[gk], np.float32).reshape(BT, K)).astype(NP_BF16)

    in_maps = []
    for c in range(NCORES):
        sl = slice(c * T, (c + 1) * T)
        xt8 = np.ascontiguousarray(
            xt_full[:, sl].reshape(4, 2, P, T).transpose(0, 2, 1, 3)
            .reshape(4 * P, 2 * T)).astype(NP_FP8)
        in_maps.append({
            "xt": np.ascontiguousarray(xt_bf[:, sl]),
            "xt8": xt8,
            "eqk8": eqk8,
            "embt_v": embt_v,
            "w8": w8,
            "w_v": w_v,
            "mt_q": _swz(np.ascontiguousarray(
                mts["q"][:, sl].astype(np.float32)), T).astype(NP_FP8),
            "mt_k": _swz(np.ascontiguousarray(
                mts["k"][:, sl].astype(np.float32)), T).astype(NP_FP8),
            "mt_v": _swz(np.ascontiguousarray(mts["v"][:, sl]), T),
            "wo": wo,
            "tri": tri,
            "par": np.full((P, 1), 1.0 if c % 2 == 0 else 0.0, np.float32),
            "z8": z8,
            "z16": z16,
        })
    return in_maps


def run(inputs, **kw):
    in_maps = prepare_in_maps(inputs)
    nc = _get_nc()
    res = run_bass_kernel_spmd(nc, in_maps, core_ids=list(range(NCORES)), **kw)
    out = np.concatenate(
        [np.asarray(r["out"], np.float32) for r in res.results], axis=0)
    return out.reshape(B, S, D), res


def kernel(**inputs):
    out, _ = run(inputs)
    return out


# revision 28
# speedup vs baseline: 1.8753x; 1.0267x over previous
"""Distributed Trainium2 Bass kernel for nn_AttentionCircuit (moe_routing).

8 NeuronCores, SPMD, fully token-sharded (T=512 tokens/core, batch b=c//2):
  Phase 1: dense A^T = emb @ x^T per pool; G^T = M^T * A^T (gates
    pre-scattered host-side); emits V[t,d] / Q^T,K^T[d,t].
    qk-side (A_qk, Q emit, K emit) runs fp8e4 DoubleRow (2 contraction
    chunks per matmul) with a host-side x16 scale on emb/w absorbed by
    the exp(score/256) trick; V side stays bf16 for accuracy.
  Phase 2: causal attention stays token-local: a core's queries only
    need the previous 512 tokens' K/V, which live on the paired core.
    Pair ReduceScatters (groups [[0,1],[2,3],..]) move K^T (fp8) and
    V-with-ones-columns (bf16) from even->odd cores; sends are
    parity-masked (x par), slot0 carries zeros, so even cores receive
    exact zeros: zero K -> exp(0)=1 but zero V AND zero ones-column
    kill both numerator and softmax denominator -- no masking ops.
    Attention runs in two passes per head (own keys first, neighbor
    keys after) so the V ReduceScatter hides under Q emit + pass A.
  Phase 3: token-local W_O projection -> out [T, D] f32.
"""

import sys

sys.path.insert(0, "/opt/trn_rl_repo")

import numpy as np
import ml_dtypes

import concourse.bass as bass
import concourse.mybir as mybir
import concourse.tile as tile
from concourse import bacc
from concourse.bass_utils import run_bass_kernel_spmd

BF16 = mybir.dt.bfloat16
FP8 = mybir.dt.float8e4
F32 = mybir.dt.float32
NP_BF16 = ml_dtypes.bfloat16
NP_FP8 = ml_dtypes.float8_e4m3
AF = mybir.ActivationFunctionType
ALU = mybir.AluOpType
DR = mybir.MatmulPerfMode.DoubleRow

B, S, D = 4, 1024, 1024
N, K = 4096, 16
H = 16
DH = D // H            # 64
NCORES = 8
BT = B * S             # 4096 tokens
T = BT // NCORES       # 512 tokens per core
P = 128
NT = N // P            # 32 n-chunks
DT_ = D // P           # 8 d-chunks
TT = T // P            # 4 token tiles per core
W65 = DH + 1           # per-head V width incl ones column
VW = H * W65           # 1040 columns per v t-chunk
SCALE = float(1.0 / np.sqrt(np.float32(DH)))


def build_nc(reps=1, stop=7):
    nc = bacc.Bacc(None, target_bir_lowering=False)

    xt = nc.declare_dram_parameter("xt", [D, T], BF16, isOutput=False)
    xt8 = nc.declare_dram_parameter("xt8", [4 * P, 2 * T], FP8, isOutput=False)
    eqk8 = nc.declare_dram_parameter("eqk8", [4 * P, 2 * N], FP8, isOutput=False)
    embt_v = nc.declare_dram_parameter("embt_v", [D, N], BF16, isOutput=False)
    w8 = nc.declare_dram_parameter("w8", [N // 2, D * 2], FP8, isOutput=False)
    w_v = nc.declare_dram_parameter("w_v", [N // 4, 4 * D], BF16, isOutput=False)
    mt_q = nc.declare_dram_parameter("mt_q", [N // 4, 4 * T], FP8, isOutput=False)
    mt_k = nc.declare_dram_parameter("mt_k", [N // 4, 4 * T], FP8, isOutput=False)
    mt_v = nc.declare_dram_parameter("mt_v", [N // 4, 4 * T], BF16, isOutput=False)
    wo = nc.declare_dram_parameter("wo", [D, D], BF16, isOutput=False)
    tri = nc.declare_dram_parameter("tri", [P, P], BF16, isOutput=False)
    par = nc.declare_dram_parameter("par", [P, 1], F32, isOutput=False)
    z8 = nc.declare_dram_parameter("z8", [P, DT_ * T], FP8, isOutput=False)
    z16 = nc.declare_dram_parameter("z16", [P, TT * VW], BF16, isOutput=False)
    out_ext = nc.declare_dram_parameter("out", [T, D], BF16, isOutput=True)

    rg2 = [[2 * i, 2 * i + 1] for i in range(NCORES // 2)]

    class StopPhase(Exception):
        pass

    with tile.TileContext(nc) as tc:
      for _rep in range(reps):
        try:
          with (
              tc.tile_pool(name="persist", bufs=1) as persist,
              tc.tile_pool(name="dram", bufs=1, space="DRAM") as dpool,
          ):
            # ---------- resident loads (gpsimd SWDGE; xt8 first: A_qk
            # needs it immediately, xt only by A_v) ----------
            tri_t = persist.tile([P, P], BF16, tag="tri", name="tri")
            nc.scalar.dma_start(out=tri_t[:], in_=tri[:, :])
            par_t = persist.tile([P, 1], F32, tag="par", name="par")
            nc.scalar.dma_start(out=par_t[:], in_=par[:, :])

            # collective buffers; slot0 = zeros (DRAM->DRAM from host zeros)
            rsk_in = dpool.tile([2, P, DT_ * T], FP8, tag="rski", name="rski")
            rsk_out = dpool.tile([P, DT_ * T], FP8, tag="rsko", name="rsko")
            rsv_in = dpool.tile([2, P, TT * VW], BF16, tag="rsvi", name="rsvi")
            rsv_out = dpool.tile([P, TT * VW], BF16, tag="rsvo", name="rsvo")
            xt_t = [persist.tile([P, T], BF16, tag=f"xt{i}", name=f"xt{i}")
                    for i in range(DT_)]
            for i in range(DT_):
                nc.gpsimd.dma_start(out=xt_t[i][:], in_=xt[i * P:(i + 1) * P, :])
            nc.gpsimd.dma_start(out=rsk_in[0, :, :], in_=z8[:, :])
            nc.gpsimd.dma_start(out=rsv_in[0, :, :], in_=z16[:, :])

            # A_qk tiles (fp8: score path tolerates the quantization)
            Abig = [persist.tile([P, 8 * T], FP8, tag=f"A{j}", name=f"A{j}")
                    for j in range(NT // 8)]

            def a_sl(nci, lo=0, width=T):
                return Abig[nci // 8][:, (nci % 8) * T + lo:
                                      (nci % 8) * T + lo + width]

            Gv = [persist.tile([P, 8 * T], BF16, tag=f"Gv{j}", name=f"Gv{j}")
                  for j in range(NT // 8)]

            def gv_sl(nci, lo=0, width=T):
                return Gv[nci // 8][:, (nci % 8) * T + lo:
                                    (nci % 8) * T + lo + width]

            def g_sl(G, nci):
                return G[nci // 8][:, (nci % 8) * T:(nci % 8 + 1) * T]

            def g_pair(G, pg):
                nci = 2 * pg
                v = G[nci // 8][:, (nci % 8) * T:(nci % 8 + 2) * T]
                return v.rearrange("p (two f) -> p two f", two=2)

            kt_big = persist.tile([P, DT_ * T], FP8, tag="kt", name="kt")
            # snd_k doubles as qt_big later (send is consumed long before
            # Q emit writes it)
            qt_big = persist.tile([P, DT_ * T], FP8, tag="qt", name="qt")
            snd_k = qt_big

            psem_cm = tc.tile_pool(name="ps_all", bufs=1, space="PSUM")
            psem = psem_cm.__enter__()

            # PE warm-up: junk matmuls keep the pstate ramp hot while the
            # first eqk/xt8 tiles stream in (PE drops to 0.65-1.2 GHz after
            # any idle and takes ~3us to re-ramp)
            wrm = persist.tile([P, T], BF16, tag="wrm", name="wrm")
            nc.vector.memset(wrm[:], 0.0)
            wps = psem.tile([P, T], F32, tag="eps7", name="wps")
            for wi in range(14):
                nc.tensor.matmul(out=wps[:], lhsT=wrm[:, 0:P], rhs=wrm[:],
                                 start=(wi == 0), stop=(wi == 13))

            # ---------- Phase 1a: A_qk (fp8 DoubleRow); eqk on sync ----------
            with tc.tile_pool(name="eqk", bufs=2) as eqkp:
                xt8_t = [eqkp.tile([P, 2 * T], FP8, tag=f"xt8{c}",
                                   name=f"xt8{c}", bufs=1) for c in range(4)]
                for c in range(4):
                    nc.scalar.dma_start(out=xt8_t[c][:],
                                        in_=xt8[c * P:(c + 1) * P, :])
                for q in range(4):
                    ech = [eqkp.tile([P, 2 * P * 8], FP8, tag=f"eq{c}",
                                     name=f"eq{c}") for c in range(4)]
                    for c in range(4):
                        nc.sync.dma_start(
                            out=ech[c][:],
                            in_=eqk8[c * P:(c + 1) * P,
                                     q * 2048:(q + 1) * 2048])
                    for nn in range(8):
                        nci = q * 8 + nn
                        ps = psem.tile([P, T], F32, tag=f"eps{nci % 8}",
                                       name="actps")
                        for c in range(4):
                            nc.tensor.matmul(
                                out=ps[:],
                                lhsT=ech[c][:, nn * 256:(nn + 1) * 256]
                                .rearrange("p (two f) -> p two f", two=2),
                                rhs=xt8_t[c][:]
                                .rearrange("p (two f) -> p two f", two=2),
                                start=(c == 0), stop=(c == 3),
                                perf_mode=DR,
                            )
                        nc.vector.tensor_copy(out=a_sl(nci), in_=ps[:])

            with tc.tile_pool(name="mstream", bufs=2) as ms:
                # w8 loaded once, resident through Q emit (8 x [P, 4096])
                w8r_cm = tc.tile_pool(name="w8r", bufs=1)
                w8r = w8r_cm.__enter__()
                w8t = [w8r.tile([P, 2 * 2 * D], FP8, tag=f"w8{i}",
                                name=f"w8{i}") for i in range(NT // 4)]
                for i in range(NT // 4):
                    for half in range(2):
                        nc.sync.dma_start(
                            out=w8t[i][:, half * 2 * D:(half + 1) * 2 * D],
                            in_=w8[(i * 2 + half) * P:
                                   (i * 2 + half + 1) * P, :])

                def w8_sl(pg, dd):
                    return w8t[pg // 2][:, (pg % 2) * 2 * D + dd * 256:
                                        (pg % 2) * 2 * D + (dd + 1) * 256]

                # ---------- Phase 1c: K emit (DoubleRow) + RS_k ----------
                # G_k pairs built on the fly (DVE idle during emit)
                with tc.tile_pool(name="gkp", bufs=4) as gkp:
                    psq = [psem.tile([P, T], F32, tag=f"eps{d}", name="kps")
                           for d in range(DT_)]
                    mk_ch = None
                    for pg in range(NT // 2):
                        if pg % 2 == 0:
                            mk_ch = ms.tile([P, 4 * T], FP8, tag="mt8",
                                            name="mt8")
                            nc.scalar.dma_start(
                                out=mk_ch[:],
                                in_=mt_k[(pg // 2) * P:(pg // 2 + 1) * P, :])
                        gk = gkp.tile([P, 2 * T], FP8, tag="gk", name="gk")
                        for k in range(2):
                            nci = pg * 2 + k
                            nc.vector.tensor_tensor(
                                out=gk[:, k * T:(k + 1) * T], in0=a_sl(nci),
                                in1=mk_ch[:, (nci % 4) * T:(nci % 4 + 1) * T],
                                op=ALU.mult)
                        for dd in range(DT_):
                            nc.tensor.matmul(
                                out=psq[dd][:],
                                lhsT=w8_sl(pg, dd)
                                .rearrange("p (two f) -> p two f", two=2),
                                rhs=gk[:]
                                .rearrange("p (two f) -> p two f", two=2),
                                start=(pg == 0), stop=(pg == NT // 2 - 1),
                                perf_mode=DR,
                            )
                    # kt evac + masked send on DVE, ahead of G_q in queue
                    for dd in range(DT_):
                        nc.vector.tensor_scalar_mul(
                            kt_big[:, dd * T:(dd + 1) * T], psq[dd][:],
                            1.0 / 16.0)
                    nc.vector.tensor_scalar_mul(snd_k[:], kt_big[:], par_t[:])
                    nc.gpsimd.dma_start(out=rsk_in[1, :, :], in_=snd_k[:])
                    nc.gpsimd.collective_compute(
                        "ReduceScatter", ALU.add, replica_groups=rg2,
                        ins=[rsk_in.opt()], outs=[rsk_out.opt()])

                # embt_v prefetch (quarters 0,1 on scalar) + A_v stream pool
                if stop < 2:
                    raise StopPhase()
                evp_cm = tc.tile_pool(name="ev", bufs=2)
                evp = evp_cm.__enter__()
                ech_v = {}
                for q in range(2):
                    ech_v[q] = [evp.tile([P, 8 * P], BF16, tag=f"ev{d}",
                                         name=f"ev{d}") for d in range(DT_)]
                    for d in range(DT_):
                        nc.scalar.dma_start(
                            out=ech_v[q][d][:],
                            in_=embt_v[d * P:(d + 1) * P,
                                       q * 1024:(q + 1) * 1024])

                # ---------- Phase 1d: A_v (bf16); evac fused with M_v
                # gating: Abig <- A_v (x) m_v = G_v directly ----------
                mv_ch = {}
                for ncg in range(NT // 4):
                    mv_ch[ncg] = ms.tile([P, 4 * T], BF16, tag="mt", name="mt")
                    nc.gpsimd.dma_start(
                        out=mv_ch[ncg][:],
                        in_=mt_v[ncg * P:(ncg + 1) * P, :])
                for q in range(4):
                    if q >= 2:   # quarters 2,3 streamed just in time
                        ech_v[q] = [evp.tile([P, 8 * P], BF16, tag=f"ev{d}",
                                             name=f"ev{d}")
                                    for d in range(DT_)]
                        for d in range(DT_):
                            nc.scalar.dma_start(
                                out=ech_v[q][d][:],
                                in_=embt_v[d * P:(d + 1) * P,
                                           q * 1024:(q + 1) * 1024])
                    for nn in range(8):
                        nci = q * 8 + nn
                        ps = psem.tile([P, T], F32, tag=f"eps{nci % 8}",
                                       name="avps")
                        for d in range(DT_):
                            nc.tensor.matmul(
                                out=ps[:],
                                lhsT=ech_v[q][d][:, nn * P:(nn + 1) * P],
                                rhs=xt_t[d][:],
                                start=(d == 0), stop=(d == DT_ - 1),
                            )
                        nc.vector.tensor_tensor(
                            out=gv_sl(nci), in0=ps[:],
                            in1=mv_ch[nci // 4][:, (nci % 4) * T:
                                                (nci % 4 + 1) * T],
                            op=ALU.mult)
                evp_cm.__exit__(None, None, None)

                # prefetch first mt_q chunks (consumed by Q emit's G_q build)
                mq_ch = {}
                for pg2 in range(2):
                    mq_ch[pg2] = ms.tile([P, 4 * T], FP8, tag="mt8",
                                         name="mt8")
                    nc.scalar.dma_start(
                        out=mq_ch[pg2][:],
                        in_=mt_q[pg2 * P:(pg2 + 1) * P, :])

                # ---------- Phase 1e: V emit + RS_v ----------
                vloc = [persist.tile([P, VW], BF16, tag=f"vl{t_}",
                                     name=f"vl{t_}") for t_ in range(TT)]
                snd_v = persist.tile([P, TT * VW], BF16, tag="sv", name="sv")
                for t_ in range(TT):
                    nc.vector.memset(
                        vloc[t_][:].rearrange(
                            "p (h f) -> p h f", f=W65)[:, :, DH:W65], 1.0)
                with tc.tile_pool(name="wvs", bufs=2) as wvp:
                    psv = [psem.tile([P, T], F32, tag=f"eps{i}", name="vps")
                           for i in range(8)]
                    for ncg in range(NT // 4):
                        wch = wvp.tile([P, 4 * D], BF16, tag="wv", name="wv")
                        nc.sync.dma_start(
                            out=wch[:], in_=w_v[ncg * P:(ncg + 1) * P, :])
                        for k in range(4):
                            nci = ncg * 4 + k
                            for tt_ in range(TT):
                                for hf in range(2):
                                    nc.tensor.matmul(
                                        out=psv[tt_ * 2 + hf][:],
                                        lhsT=gv_sl(nci, lo=tt_ * P, width=P),
                                        rhs=wch[:, k * D + hf * T:
                                                k * D + (hf + 1) * T],
                                        start=(nci == 0),
                                        stop=(nci == NT - 1),
                                    )
                    for tt_ in range(TT):
                        for hf in range(2):
                            dst = (vloc[tt_][:, hf * 8 * W65:
                                             (hf + 1) * 8 * W65]
                                   .rearrange("p (h f) -> p h f", f=W65)
                                   [:, :, 0:DH])
                            srcv = (psv[tt_ * 2 + hf][:]
                                    .rearrange("p (h f) -> p h f", f=DH))
                            if (tt_ * 2 + hf) % 2 == 0:
                                nc.scalar.activation(out=dst, in_=srcv,
                                                     func=AF.Copy)
                            else:
                                nc.vector.tensor_copy(out=dst, in_=srcv)
                    for tt_ in range(TT):
                        nc.vector.tensor_scalar_mul(
                            snd_v[:, tt_ * VW:(tt_ + 1) * VW],
                            vloc[tt_][:], par_t[:])
                    nc.gpsimd.dma_start(out=rsv_in[1, :, :], in_=snd_v[:])
                    nc.gpsimd.collective_compute(
                        "ReduceScatter", ALU.add, replica_groups=rg2,
                        ins=[rsv_in.opt()], outs=[rsv_out.opt()])

                # ---------- Phase 1f: Q emit (DoubleRow, resident w8;
                # G_q pairs built on the fly: DVE is idle here) ----------
                if stop < 3:
                    raise StopPhase()
                with tc.tile_pool(name="gqp", bufs=4) as gqp:
                    psq = [psem.tile([P, T], F32, tag=f"eps{d}", name="qps")
                           for d in range(DT_)]
                    for pg in range(NT // 2):
                        mg = pg // 2
                        if pg % 2 == 0 and mg not in mq_ch:
                            mq_ch[mg] = ms.tile([P, 4 * T], FP8, tag="mt8",
                                                name="mt8")
                            nc.scalar.dma_start(
                                out=mq_ch[mg][:],
                                in_=mt_q[mg * P:(mg + 1) * P, :])
                        gq = gqp.tile([P, 2 * T], FP8, tag="gq", name="gq")
                        for k in range(2):
                            nci = pg * 2 + k
                            nc.vector.tensor_tensor(
                                out=gq[:, k * T:(k + 1) * T], in0=a_sl(nci),
                                in1=mq_ch[mg][:, (nci % 4) * T:
                                               (nci % 4 + 1) * T],
                                op=ALU.mult)
                        for dd in range(DT_):
                            nc.tensor.matmul(
                                out=psq[dd][:],
                                lhsT=w8_sl(pg, dd)
                                .rearrange("p (two f) -> p two f", two=2),
                                rhs=gq[:]
                                .rearrange("p (two f) -> p two f", two=2),
                                start=(pg == 0), stop=(pg == NT // 2 - 1),
                                perf_mode=DR,
                            )
                    for dd in range(DT_):
                        nc.vector.tensor_scalar_mul(
                            qt_big[:, dd * T:(dd + 1) * T], psq[dd][:],
                            1.0 / 16.0)
                w8r_cm.__exit__(None, None, None)

            # receive DMAs late on sync (after w_v drained; data long ready)
            krecv = persist.tile([P, DT_ * T], FP8, tag="kr", name="kr")
            vrecv = persist.tile([P, TT * VW], BF16, tag="vr", name="vr")
            nc.sync.dma_start(out=krecv[:], in_=rsk_out[:, :])
            nc.sync.dma_start(out=vrecv[:], in_=rsv_out[:, :])

            psem_cm.__exit__(None, None, None)

            # ---------- Phase 2: token-local causal attention ----------
            def kt_sl(src, h, j):
                return src[(h % 2) * DH:(h % 2 + 1) * DH,
                           (h // 2) * T + j * P:(h // 2) * T + (j + 1) * P]

            def qt_sl(h, lo, width):
                return qt_big[(h % 2) * DH:(h % 2 + 1) * DH,
                              (h // 2) * T + lo:(h // 2) * T + lo + width]

            aout = persist.tile([P, DT_ * T], BF16, tag="aout", name="aout")
            pvA = [persist.tile([W65, T], BF16, tag=f"pvA{h}", name=f"pvA{h}")
                   for h in range(H)]

            with (
                tc.tile_pool(name="attn", bufs=2) as attnp,
                tc.tile_pool(name="ptp", bufs=8) as ptp,
                tc.tile_pool(name="wop", bufs=1) as wop,
            ):
                wo_t = [wop.tile([P, D], BF16, tag=f"wo{dd}", name=f"wo{dd}")
                        for dd in range(DT_)]
                for dd in range(DT_):
                    nc.gpsimd.dma_start(out=wo_t[dd][:],
                                        in_=wo[dd * P:(dd + 1) * P, :])

                # pass A: own keys (causal)
                if stop < 4:
                    raise StopPhase()
                psatt_cm = tc.tile_pool(name="psatt", bufs=4, space="PSUM")
                pss = psatt_cm.__enter__()
                pspv_cm = tc.tile_pool(name="pspv", bufs=2, space="PSUM")
                pspv = pspv_cm.__enter__()
                ptsA = {}
                for h in range(H + 1):
                    if h < H:
                        pts = []
                        for jo in range(TT):
                            lo = jo * P
                            ps_s = pss.tile([P, T], F32, tag="s", name="s")
                            nc.tensor.matmul(
                                out=ps_s[:, lo:T],
                                lhsT=kt_sl(kt_big, h, jo),
                                rhs=qt_sl(h, lo, T - lo),
                                start=True, stop=True)
                            pt = ptp.tile([P, T], BF16, tag="pt", name="pt")
                            if lo > 0:
                                nc.vector.memset(pt[:, 0:lo], 0.0)
                            nc.scalar.activation(
                                out=pt[:, lo:T], in_=ps_s[:, lo:T],
                                func=AF.Exp, scale=SCALE / 256.0)
                            nc.vector.tensor_tensor(
                                out=pt[:, lo:lo + P], in0=pt[:, lo:lo + P],
                                in1=tri_t[:], op=ALU.mult)
                            pts.append(pt)
                        ptsA[h] = pts
                    if h >= 1:
                        hp = h - 1
                        pv = pspv.tile([W65, T], F32, tag="pv", name="pv")
                        for jo in range(TT):
                            nc.tensor.matmul(
                                out=pv[:],
                                lhsT=vloc[jo][:, hp * W65:(hp + 1) * W65],
                                rhs=ptsA[hp][jo][:],
                                start=(jo == 0), stop=(jo == TT - 1))
                        nc.vector.tensor_copy(out=pvA[hp][:], in_=pv[:])
                        del ptsA[hp]

                # pass B: neighbor keys (zeros on even cores) + normalize
                if stop < 5:
                    pspv_cm.__exit__(None, None, None)
                    psatt_cm.__exit__(None, None, None)
                    raise StopPhase()
                ptsB = {}
                for h in range(H + 1):
                    if h < H:
                        pts = []
                        for jn in range(TT):
                            ps_s = pss.tile([P, T], F32, tag="s", name="s")
                            nc.tensor.matmul(
                                out=ps_s[:],
                                lhsT=kt_sl(krecv, h, jn),
                                rhs=qt_sl(h, 0, T),
                                start=True, stop=True)
                            pt = ptp.tile([P, T], BF16, tag="pt", name="pt")
                            nc.scalar.activation(
                                out=pt[:], in_=ps_s[:],
                                func=AF.Exp, scale=SCALE / 256.0)
                            pts.append(pt)
                        ptsB[h] = pts
                    if h < 1:
                        continue
                    h_ = h - 1
                    pv2 = pspv.tile([W65, T], F32, tag="pv2", name="pv2")
                    for jn in range(TT):
                        nc.tensor.matmul(
                            out=pv2[:],
                            lhsT=vrecv[:, jn * VW + h_ * W65:
                                       jn * VW + (h_ + 1) * W65],
                            rhs=ptsB[h_][jn][:],
                            start=(jn == 0), stop=(jn == TT - 1))
                    del ptsB[h_]
                    h = h_
                    pvc = attnp.tile([W65, T], BF16, tag="pvc", name="pvc")
                    if stop < 6:
                        nc.scalar.activation(out=pvc[:], in_=pv2[:],
                                             func=AF.Copy)
                        continue
                    nc.vector.tensor_tensor(out=pvc[:], in0=pv2[:],
                                            in1=pvA[h][:], op=ALU.add)
                    rec = attnp.tile([1, T], BF16, tag="rec", name="rec")
                    with nc.allow_low_precision(
                            reason="softmax denom recip in bf16"):
                        nc.vector.reciprocal(out=rec[:],
                                             in_=pvc[DH:W65, :])
                    bc_sb = attnp.tile([DH, T], BF16, tag="bc", name="bc")
                    nc.gpsimd.partition_broadcast(bc_sb[:], rec[:])
                    att_sb = attnp.tile([DH, T], BF16, tag="att", name="att")
                    nc.vector.tensor_tensor(
                        out=att_sb[:], in0=pvc[0:DH, :], in1=bc_sb[:],
                        op=ALU.mult)
                    nc.gpsimd.dma_start(
                        out=aout[(h % 2) * DH:(h % 2 + 1) * DH,
                                 (h // 2) * T:(h // 2 + 1) * T],
                        in_=att_sb[:])

                pspv_cm.__exit__(None, None, None)
                psatt_cm.__exit__(None, None, None)

                # ---------- Phase 3: W_O ----------
                if stop < 7:
                    raise StopPhase()
                with tc.tile_pool(name="pswo", bufs=1, space="PSUM") as pswo:
                    pso = [pswo.tile([P, T], F32, tag=f"wops{i}",
                                     name=f"wops{i}") for i in range(8)]
                    for dd in range(DT_):
                        for tt_ in range(TT):
                            for hf in range(2):
                                nc.tensor.matmul(
                                    out=pso[tt_ * 2 + hf][:],
                                    lhsT=aout[:, dd * T + tt_ * P:
                                              dd * T + (tt_ + 1) * P],
                                    rhs=wo_t[dd][:, hf * T:(hf + 1) * T],
                                    start=(dd == 0), stop=(dd == DT_ - 1),
                                )
                            if dd == DT_ - 1:
                                # evac+ship right behind each group's stop
                                out_sb = wop.tile([P, D], BF16, tag="osb",
                                                  name="osb", bufs=4)
                                nc.scalar.activation(
                                    out=out_sb[:, 0:T],
                                    in_=pso[tt_ * 2][:], func=AF.Copy)
                                nc.vector.tensor_copy(
                                    out=out_sb[:, T:2 * T],
                                    in_=pso[tt_ * 2 + 1][:])
                                nc.sync.dma_start(
                                    out=out_ext[tt_ * P:(tt_ + 1) * P, :],
                                    in_=out_sb[:])

        except StopPhase:
            pass

    nc.finalize()
    return nc


_NC_CACHE = {}


def _get_nc():
    if "nc" not in _NC_CACHE:
        _NC_CACHE["nc"] = build_nc()
    return _NC_CACHE["nc"]


def _scatter_gates(idx, gate):
    """[N, BT] matrix M^T with M^T[n, t] = sum_k gate[t,k]*(idx[t,k]==n)."""
    mt = np.zeros((N, BT), np.float32)
    t_idx = np.repeat(np.arange(BT, dtype=np.int64), K)
    np.add.at(mt, (idx.reshape(-1).astype(np.int64), t_idx),
              gate.reshape(-1))
    return mt


def _swz(w, cols):
    return np.ascontiguousarray(
        w.reshape(N // 512, 4, P, cols).transpose(0, 2, 1, 3)
        .reshape(N // 4, 4 * cols))


def prepare_in_maps(inputs):
    x = np.asarray(inputs["x"], np.float32).reshape(BT, D)
    xt_full = np.ascontiguousarray(x.T)                      # [D, BT] f32
    xt_bf = xt_full.astype(NP_BF16)

    emb16 = np.asarray(inputs["qk_emb"], np.float32).T * 16.0  # [D, N]
    eqk8 = np.ascontiguousarray(
        emb16.reshape(4, 2, P, NT, P).transpose(0, 2, 3, 1, 4)
        .reshape(4 * P, 2 * N)).astype(NP_FP8)
    w16 = np.asarray(inputs["qk_w"], np.float32) * 16.0       # [N, D]
    w8 = np.ascontiguousarray(
        w16.reshape(N // 256, 2, P, DT_, P).transpose(0, 2, 3, 1, 4)
        .reshape(N // 2, 2 * D)).astype(NP_FP8)

    embt_v = np.ascontiguousarray(
        np.asarray(inputs["v_emb"], np.float32).T).astype(NP_BF16)
    w_v = _swz(np.asarray(inputs["v_w"], np.float32), D).astype(NP_BF16)
    wo = np.asarray(inputs["W_O"], np.float32).astype(NP_BF16)
    tri = np.triu(np.ones((P, P), np.float32)).astype(NP_BF16)
    z8 = np.zeros((P, DT_ * T), NP_FP8)
    z16 = np.zeros((P, TT * VW), NP_BF16)

    mts = {}
    for side, gk, ik in (("q", "tk_g_Q", "tk_i_Q"),
                         ("k", "tk_g_K", "tk_i_K"),
                         ("v", "tk_g_V", "tk_i_V")):
        mts[side] = _scatter_gates(
            np.asarray(inputs[ik]).reshape(BT, K),
            np.asarray(inputs[gk], np.float32).reshape(BT, K)).astype(NP_BF16)

    in_maps = []
    for c in range(NCORES):
        sl = slice(c * T, (c + 1) * T)
        xt8 = np.ascontiguousarray(
            xt_full[:, sl].reshape(4, 2, P, T).transpose(0, 2, 1, 3)
            .reshape(4 * P, 2 * T)).astype(NP_FP8)
        in_maps.append({
            "xt": np.ascontiguousarray(xt_bf[:, sl]),
            "xt8": xt8,
            "eqk8": eqk8,
            "embt_v": embt_v,
            "w8": w8,
            "w_v": w_v,
            "mt_q": _swz(np.ascontiguousarray(
                mts["q"][:, sl].astype(np.float32)), T).astype(NP_FP8),
            "mt_k": _swz(np.ascontiguousarray(
                mts["k"][:, sl].astype(np.float32)), T).astype(NP_FP8),
            "mt_v": _swz(np.ascontiguousarray(mts["v"][:, sl]), T),
            "wo": wo,
            "tri": tri,
            "par": np.full((P, 1), 1.0 if c % 2 == 0 else 0.0, np.float32),
            "z8": z8,
            "z16": z16,
        })
    return in_maps


def run(inputs, **kw):
    in_maps = prepare_in_maps(inputs)
    nc = _get_nc()
    res = run_bass_kernel_spmd(nc, in_maps, core_ids=list(range(NCORES)), **kw)
    out = np.concatenate(
        [np.asarray(r["out"], np.float32) for r in res.results], axis=0)
    return out.reshape(B, S, D), res


def kernel(**inputs):
    out, _ = run(inputs)
    return out
